# revision 41
# baseline (speedup 1.0000x reference)
"""GATNet (3-layer GAT + final linear) on 8 Trainium2 NeuronCores via Bass.

Graph/data-parallel layout (per sharding hint):
  - Nodes sharded by dst across 8 cores (6250/core).  Every core keeps a full
    replica of hA_l = [h_l | alpha_src_l] (bf16, rows padded to 256B stride)
    in DRAM; per-edge features are fetched with batched GPSIMD dma_gather
    (int16 indices, table split at row 32768 into lo/hi halves).
  - Per core, edges are grouped into B blocks (<=128 dst nodes each,
    <=LO_CAP lo-edges + <=HI_CAP hi-edges, fixed CH=LO+HI chunks of 128 edge
    slots).  The one-hot S [e x n] is built on device from per-slot dst ids
    (dstloc == iota); S^T for all blocks is precomputed once at setup (per
    2-block group: equality + 128x128 XBAR DMA transposes, streamed to
    DRAM) so the block loop needs a single DMA per block instead of a
    24-instruction transpose chain (the block loop is latency-bound at
    ~10us per dependent instruction, not bandwidth-bound).
    Aggregation = PE matmul psum[n,:] += S_j^T @ msg_j, where
    msg = [h[src]*exp(e) | exp(e)] so the same matmul yields the softmax
    denominator; normalization happens after aggregation (linearity).
  - alpha_dst broadcast to edges via S^T-matmul with the block's alpha_dst
    rows (fetched batched from the x-tables; the layer-0 table holds only
    the 8 alpha_dst_1 columns - raw x is never read through the gather).
  - Layer transition: x_l(shard) -> XBAR-transpose -> matmul with
    Wa = [W | W@a_src | W@a_dst]; AllGather(shard) -> full hA replica
    (Shared address space for the HBM-HBM collective fast path).
    Layer-1 hA is computed the same way from a per-core x^T shard; the
    alpha_dst_1 column of the x0 table is filled by the same loop.
  - Final linear+sigmoid fused into layer-3 epilogue; host concatenates the
    per-core [6250,1] output shards.

Host-side cost of the per-call path is minimized because the axon tunnel
moves ~75 MB/s and re-runs jax lowering per call:
  - payload is ~0.7 MB/core: gather indices ship as [16, .] int16 and are
    partition-replicated on device; W2/W3 ship row-sharded and are
    AllGathered; wf/bias/iota rows ship as [1, .] and are partition-doubled
    on device via SBUF->SBUF DMA; no dense one-hot matrices ship at all.
  - the persistent XLA compilation cache is enabled (else every call pays
    a ~1.2 s walrus re-verify) and the immutable post-compile BIR JSON is
    cached on the nc object (else every call re-serializes 20k
    instructions, ~0.15 s).
"""

import os
import numpy as np
import ml_dtypes

import jax

# Persistent XLA compilation cache: without it every run_bass_kernel_spmd
# call re-runs the walrus BIR verify/NEFF-packaging subprocess (~1.2 s).
for _k, _v in (("jax_compilation_cache_dir", "/tmp/jax_comp_cache"),
               ("jax_persistent_cache_min_compile_time_secs", 0),
               ("jax_persistent_cache_min_entry_size_bytes", -1)):
    try:
        jax.config.update(_k, _v)
    except Exception:
        pass

from concourse import bass, mybir, bacc
import concourse.tile as tile
from concourse import bass_utils
from concourse.masks import make_identity

BF16 = ml_dtypes.bfloat16
NEG_SLOPE = 0.2
EPS = 1e-16
I16_SPLIT = 32768


def rup(x, m):
    return (x + m - 1) // m * m


# ---------------------------------------------------------------- config ----


class Cfg:
    def __init__(self, N, ncores, layers, lo_chunks, hi_chunks):
        self.N = N
        self.ncores = ncores
        self.shard = N // ncores
        assert self.shard * ncores == N
        self.layers = layers                       # [(Fin, H, C)]
        self.loch, self.hich = lo_chunks, hi_chunks
        self.chunks = lo_chunks + hi_chunks
        self.lo_cap = lo_chunks * 128
        self.hi_cap = hi_chunks * 128
        self.losplit = I16_SPLIT if N > I16_SPLIT else N // 2
        self.Fs = [H * C for (_, H, C) in layers]
        self.Hs = [H for (_, H, C) in layers]
        self.rowws = [F + H for F, H in zip(self.Fs, self.Hs)]
        self.rowps = [rup(r, 128) for r in self.rowws]      # padded hA rows
        # x tables: [x_l | alpha_dst_{l+1}]; x0 table holds only alpha_dst_1
        # (the raw x columns are never read through the gather path)
        self.xrows = [self.Hs[0],
                      self.Fs[0] + self.Hs[1], self.Fs[1] + self.Hs[2]]
        self.xrowps = [rup(r, 128) for r in self.xrows]


REAL_CFG = Cfg(50000, 8, [(16, 8, 32), (256, 8, 32), (256, 12, 64)], 8, 4)


# ---------------------------------------------------------- host planning ----


def wrap16(vals, cap):
    """int16 idx stream -> wrapped [16, cap//16] layout (16 partitions).

    The gpsimd gather wants this replicated across all 8 16-partition
    groups; the replication happens on device (8 DMA loads) to keep the
    host->device payload small."""
    assert len(vals) == cap and cap % 16 == 0
    return np.asarray(vals, np.int16).reshape(cap // 16, 16).T  # [16, cap/16]


def make_plan(cfg, edge_index):
    N, shard, CH = cfg.N, cfg.shard, cfg.chunks
    src = np.concatenate([edge_index[0].astype(np.int64), np.arange(N)])
    dst = np.concatenate([edge_index[1].astype(np.int64), np.arange(N)])
    order = np.argsort(dst, kind="stable")
    src, dst = src[order].astype(np.int64), dst[order].astype(np.int64)

    bounds = np.searchsorted(dst, np.arange(0, N + 1, shard))
    is_lo = src < cfg.losplit
    deg_lo = np.bincount(dst[is_lo], minlength=N)
    deg_hi = np.bincount(dst[~is_lo], minlength=N)

    # greedy per-core blocks
    per_core_blocks = []
    for c in range(cfg.ncores):
        blocks, n = [], 0
        while n < shard:
            n_end, lo, hi = n, 0, 0
            while n_end < shard and n_end - n < 128:
                g = c * shard + n_end
                if lo + deg_lo[g] > cfg.lo_cap or hi + deg_hi[g] > cfg.hi_cap:
                    assert n_end > n, "single node exceeds caps"
                    break
                lo += deg_lo[g]
                hi += deg_hi[g]
                n_end += 1
            blocks.append((n, n_end))
            n = n_end
        per_core_blocks.append(blocks)
    B = max(len(b) for b in per_core_blocks)

    plan = {"B": B, "cores": []}
    for c in range(cfg.ncores):
        blocks = per_core_blocks[c] + \
            [(shard, shard)] * (B - len(per_core_blocks[c]))
        e0, e1 = bounds[c], bounds[c + 1]
        csrc = src[e0:e1]
        cdstl = dst[e0:e1] - c * shard
        node_starts = np.searchsorted(cdstl, np.arange(shard + 1))

        colw = CH * 128 // 16
        idx16 = np.zeros((16, B * colw), np.int16)
        nd16 = np.zeros((16, B * 8), np.int16)
        nodeidx = np.full((128, B), shard, np.int32)
        dl_int = np.full((128, B * CH), 128, np.int32)
        gcnt = np.full((1, 2 * B), 1, np.int32)
        for b, (n0, n1) in enumerate(blocks):
            nn = n1 - n0
            if nn > 0:
                nodeidx[:nn, b] = np.arange(n0, n1)
            ndstream = np.full(128, shard, np.int64)
            ndstream[:nn] = np.arange(n0, n1)
            nd16[:, b * 8:(b + 1) * 8] = wrap16(ndstream, 128)
            es, ee = node_starts[n0], node_starts[n1]
            bsrc, bdstl = csrc[es:ee], cdstl[es:ee]
            blo = bsrc < cfg.losplit
            lo_src, lo_dst = bsrc[blo], bdstl[blo]
            hi_src, hi_dst = bsrc[~blo] - cfg.losplit, bdstl[~blo]
            assert len(lo_src) <= cfg.lo_cap and len(hi_src) <= cfg.hi_cap
            lo_stream = np.full(cfg.lo_cap, -1, np.int64)
            lo_stream[:len(lo_src)] = lo_src
            hi_stream = np.full(cfg.hi_cap, -1, np.int64)
            hi_stream[:len(hi_src)] = hi_src
            if len(lo_src) == 0:
                lo_stream[0] = 0
            if len(hi_src) == 0:
                hi_stream[0] = 0
            gcnt[0, 2 * b] = max(len(lo_src), 1)
            gcnt[0, 2 * b + 1] = max(len(hi_src), 1)
            idx16[:, b * colw: b * colw + cfg.lo_cap // 16] = \
                wrap16(lo_stream, cfg.lo_cap)
            idx16[:, b * colw + cfg.lo_cap // 16:(b + 1) * colw] = \
                wrap16(hi_stream, cfg.hi_cap)
            # S: slot i -> (partition i%128, chunk i//128); built on device
            for sdst, base in [(lo_dst, 0), (hi_dst, cfg.lo_cap)]:
                ne = len(sdst)
                if ne == 0:
                    continue
                i = base + np.arange(ne)
                p, ch = i % 128, i // 128
                nl = (sdst - n0).astype(np.int64)
                dl_int[p, b * CH + ch] = nl
        plan["cores"].append(
            dict(idx16=idx16, nd16=nd16, nodeidx=nodeidx, gcnt=gcnt,
                 dstloc8=dl_int.astype(np.uint8)))
    return plan


def fold_weights(W, a_s, a_d, H, C):
    F = H * C
    Wr = np.asarray(W, np.float32).reshape(-1, H, C)
    ws = np.einsum("fhc,hc->fh", Wr, np.asarray(a_s, np.float32))
    wd = np.einsum("fhc,hc->fh", Wr, np.asarray(a_d, np.float32))
    return np.concatenate([Wr.reshape(Wr.shape[0], -1), ws, wd], axis=1)


def seg_split(total):
    segs, o = [], 0
    while o < total:
        w = min(512, total - o)
        segs.append((o, w))
        o += w
    return segs


# ------------------------------------------------------------ bass program ----


def build_nc(cfg, B):
    CH, N, shard = cfg.chunks, cfg.N, cfg.shard
    LOCH = cfg.loch
    dt = mybir.dt
    f32, bf16, i16, i32 = dt.float32, dt.bfloat16, dt.int16, dt.int32
    colw = CH * 128 // 16
    Bh = 8                     # blocks per fetch segment (<=1024 idxs/call)

    nc = bacc.Bacc("TRN2", target_bir_lowering=False, debug=False,
                   enable_asserts=False, num_devices=cfg.ncores)

    # ---- I/O ----
    Fin1 = cfg.layers[0][0]
    FTOT = sum(cfg.Fs)
    wrows = [cfg.layers[li][0] // cfg.ncores for li in range(3)]
    xTs = nc.dram_tensor("xTs", [Fin1, shard], bf16, kind="ExternalInput")
    # [Wa0 flat | Wa1 row-shard flat | Wa2 row-shard flat]
    wa_ws = [cfg.Fs[li] + 2 * cfg.Hs[li] for li in range(3)]
    wa_rs = [cfg.layers[0][0], wrows[1], wrows[2]]
    wa_off = [0, wa_rs[0] * wa_ws[0],
              wa_rs[0] * wa_ws[0] + wa_rs[1] * wa_ws[1]]
    wflat_in = nc.dram_tensor(
        "wflat", [1, wa_off[2] + wa_rs[2] * wa_ws[2]], bf16,
        kind="ExternalInput")
    # one packed row input: [wf | brep0 | brep1 | brep2 | iota]
    o_b0 = FTOT
    o_b1 = o_b0 + cfg.Fs[0]
    o_b2 = o_b1 + cfg.Fs[1]
    o_io = o_b2 + cfg.Fs[2]
    rows_in = nc.dram_tensor("rows", [1, o_io + 130], bf16,
                             kind="ExternalInput")
    idxall_in = nc.dram_tensor("idxall", [16, B * colw + B * 8], i16,
                               kind="ExternalInput")
    nodeidx_in = nc.dram_tensor("nodeidx", [128, B], i32, kind="ExternalInput")
    gcnt_in = nc.dram_tensor("gcnt", [1, 2 * B], i32, kind="ExternalInput")
    dstloc8_in = nc.dram_tensor("dstloc8", [128, B * CH], dt.uint8,
                                kind="ExternalInput")
    out = nc.dram_tensor("out", [shard, 1], f32, kind="ExternalOutput")

    # ---- internal DRAM ----
    Wstg = [None] + [nc.dram_tensor(f"Wa{li}s",
                                    [wrows[li], cfg.Fs[li] + 2 * cfg.Hs[li]],
                                    bf16, kind="Internal") for li in (1, 2)]
    Was = [None] + [nc.dram_tensor(f"Wa{li}f",
                                     [cfg.layers[li][0],
                                      cfg.Fs[li] + 2 * cfg.Hs[li]], bf16,
                                     kind="Internal", addr_space="Shared")
                      for li in (1, 2)]
    hA_full = [nc.dram_tensor(f"hAfull{li}", [N, cfg.rowps[li]], bf16,
                              kind="Internal", addr_space="Shared")
               for li in range(3)]
    hA_shard = [nc.dram_tensor(f"hAshard{li}", [shard, cfg.rowps[li]],
                               bf16, kind="Internal") for li in range(3)]
    xtab = [nc.dram_tensor(f"xtab{li}", [shard + 6, cfg.xrowps[li]],
                           bf16, kind="Internal") for li in range(3)]
    ST_dram = nc.dram_tensor("STd", [128, B * CH * 128], bf16,
                             kind="Internal")
    out_buf = nc.dram_tensor("out_buf", [shard + 1, 1], f32, kind="Internal")

    with tile.TileContext(nc) as tc:
        with tc.tile_pool(name="const", bufs=1) as cpool, \
             tc.tile_pool(name="io", bufs=3) as iop, \
             tc.tile_pool(name="gath", bufs=2) as gp, \
             tc.tile_pool(name="fetch", bufs=1) as fp, \
             tc.tile_pool(name="work", bufs=2) as wp, \
             tc.tile_pool(name="small", bufs=3) as sp, \
             tc.tile_pool(name="stsetup", bufs=2) as stp, \
             tc.tile_pool(name="psum", bufs=2, space="PSUM") as pp:


            ident = cpool.tile([128, 128], bf16)
            make_identity(nc, ident[:])

            # assemble full W2/W3 from the row shards (1/8 upload each);
            # collectives cannot read IO tensors, so stage via Internal DRAM
            wbase = wflat_in[:, :]
            for li in (1, 2):
                src = bass.AP(wbase.tensor, wa_off[li],
                              [[wa_ws[li], wa_rs[li]], [1, wa_ws[li]]])
                nc.sync.dma_start(out=Wstg[li][:, :], in_=src)
                nc.gpsimd.collective_compute(
                    "AllGather", mybir.AluOpType.bypass,
                    replica_groups=[list(range(cfg.ncores))],
                    ins=[Wstg[li][:]], outs=[Was[li][:]])

            def bcast_rows(t, row_in, width):
                """fill t[0:128, :width] with row_in via log2 partition
                doubling (SBUF->SBUF DMA)."""
                nc.sync.dma_start(out=t[0:1, :width], in_=row_in)
                p = 1
                while p < 128:
                    nc.sync.dma_start(out=t[p:2 * p, :width],
                                      in_=t[0:p, :width])
                    p *= 2

            wa_sb = []
            for li in range(3):
                Fin, H = cfg.layers[li][0], cfg.Hs[li]
                tiles = []
                for f0 in range(0, Fin, 128):
                    w = min(128, Fin - f0)
                    t = cpool.tile([128, cfg.Fs[li] + 2 * H], bf16,
                                   tag=f"wa{li}_{f0}", name=f"wa{li}_{f0}")
                    if li == 0:
                        src = bass.AP(wbase.tensor, wa_off[0] + f0 * wa_ws[0],
                                      [[wa_ws[0], w], [1, wa_ws[0]]])
                        nc.sync.dma_start(out=t[:w], in_=src)
                    else:
                        nc.sync.dma_start(out=t[:w],
                                          in_=Was[li][f0:f0 + w, :])
                    tiles.append((t, w))
                wa_sb.append(tiles)
            brep_sb = []
            boffs = [o_b0, o_b1, o_b2]
            for li in range(3):
                t = cpool.tile([128, cfg.Fs[li]], bf16, tag=f"brep{li}",
                               name=f"brepsb{li}")
                bcast_rows(t, rows_in[0:1, boffs[li]:boffs[li] + cfg.Fs[li]],
                           cfg.Fs[li])
                brep_sb.append(t)
            wf_sb = cpool.tile([128, FTOT], bf16)
            bcast_rows(wf_sb, rows_in[0:1, 0:FTOT], FTOT)
            bf_sb = cpool.tile([128, 1], f32)
            bcast_rows(bf_sb,
                       rows_in[0:1, o_io + 128:o_io + 130].bitcast(f32), 1)
            iota_sb = cpool.tile([128, 128], bf16)
            bcast_rows(iota_sb, rows_in[0:1, o_io:o_io + 128], 128)
            idx_sb = cpool.tile([128, B * colw], i16)
            nd_sb = cpool.tile([128, B * 8], i16)
            for g in range(8):
                nc.sync.dma_start(out=idx_sb[16 * g:16 * (g + 1), :],
                                  in_=idxall_in[:, :B * colw])
                nc.sync.dma_start(out=nd_sb[16 * g:16 * (g + 1), :],
                                  in_=idxall_in[:, B * colw:])
            nidx_sb = cpool.tile([128, B], i32)
            nc.sync.dma_start(out=nidx_sb[:], in_=nodeidx_in[:, :])
            dl8_sb = cpool.tile([128, B * CH], dt.uint8)
            nc.sync.dma_start(out=dl8_sb[:], in_=dstloc8_in[:, :])
            dstloc_sb = cpool.tile([128, B * CH], bf16)
            nc.vector.tensor_copy(out=dstloc_sb[:], in_=dl8_sb[:])
            gcnt_sb = cpool.tile([1, 2 * B], i32)
            nc.sync.dma_start(out=gcnt_sb[:], in_=gcnt_in[:, :])
            r_lo = nc.gpsimd.alloc_register("r_lo")
            r_hi = nc.gpsimd.alloc_register("r_hi")

            # build S^T for every block once: S = (dstloc == iota) per
            # block, 128x128 XBAR DMA transposes, stream to DRAM; double-
            # buffered so group g+1's equality overlaps g's transposes
            for q0 in range(B):
                Sq = stp.tile([128, CH * 128], bf16, tag="Sq")
                iota_q = bass.AP(iota_sb[:].tensor, iota_sb[:].offset,
                                 [iota_sb[:].ap[0], [0, CH], [1, 128]])
                nc.vector.tensor_tensor(
                    out=Sq[:].rearrange("p (c n) -> p c n", c=CH),
                    in0=dstloc_sb[:, q0 * CH:(q0 + 1) * CH]
                    .to_broadcast([128, CH, 128]),
                    in1=iota_q, op=mybir.AluOpType.is_equal)
                STq = stp.tile([128, CH * 128], bf16, tag="STq")
                for j in range(CH):
                    nc.sync.dma_start(out=STq[:, j * 128:(j + 1) * 128],
                                      in_=Sq[:, j * 128:(j + 1) * 128],
                                      transpose=True)
                nc.sync.dma_start(
                    out=ST_dram[:, q0 * CH * 128:(q0 + 1) * CH * 128],
                    in_=STq[:])

            # zero dummy rows of internal x tables
            zrow = cpool.tile([1, 512], bf16)
            nc.vector.memset(zrow[:], 0.0)
            for li in (0, 1, 2):
                nc.sync.dma_start(out=xtab[li][shard:shard + 1, :],
                                  in_=zrow[:1, :cfg.xrowps[li]])

            # ------- phase B1: hA1 = x @ Wa1 for own shard + AllGather -------
            roww0 = cfg.rowws[0]
            H1 = cfg.Hs[0]
            wa1_t = wa_sb[0][0][0]
            for t0 in range(0, shard, 128):
                w = min(128, shard - t0)
                lhs = iop.tile([Fin1, 128], bf16, tag="b1lhs")
                if w < 128:
                    nc.vector.memset(lhs[:], 0.0)
                nc.sync.dma_start(out=lhs[:, :w], in_=xTs[:, t0:t0 + w])
                ph = pp.tile([128, cfg.Fs[0] + 2 * cfg.Hs[0]], f32, tag="agg0")
                nc.tensor.matmul(out=ph[:], lhsT=lhs[:], rhs=wa1_t[:Fin1],
                                 start=True, stop=True)
                hcp = iop.tile([128, roww0], bf16, tag="b1h")
                nc.vector.tensor_copy(out=hcp[:w], in_=ph[:w, :roww0])
                nc.sync.dma_start(out=hA_shard[0][t0:t0 + w, :roww0],
                                  in_=hcp[:w])
                acp = sp.tile([128, H1], bf16, tag="bacp")
                nc.vector.tensor_copy(out=acp[:w],
                                      in_=ph[:w, roww0:roww0 + H1])
                nc.sync.dma_start(out=xtab[0][t0:t0 + w, 0:H1], in_=acp[:w])
            if not os.environ.get("GAT_NO_AG"):
                nc.gpsimd.collective_compute(
                    "AllGather", mybir.AluOpType.bypass,
                    replica_groups=[list(range(cfg.ncores))],
                    ins=[hA_shard[0][:]], outs=[hA_full[0][:]])

            # ---------------- layers ----------------------------------------
            _abl = set(os.environ.get("GAT_ABL", "").split(","))
            _maxl = int(os.environ.get("GAT_LAYERS", "3"))
            for li in range(_maxl):
                Fin, H, C = cfg.layers[li]
                F, rowp = cfg.Fs[li], cfg.rowps[li]
                segs = seg_split(F + H)
                is_last = li == 2
                adcol = 0 if li == 0 else cfg.Fs[li - 1]     # alpha_dst col
                xrowp = cfg.xrowps[li]

                for half in range((B + Bh - 1) // Bh):
                    b0 = half * Bh
                    nb = min(Bh, B - b0)
                    if nb <= 0:
                        continue
                    xad = fp.tile([128, Bh, xrowp], bf16, tag="xad")
                    nc.gpsimd.dma_gather(
                        xad[:, :nb, :], xtab[li][:],
                        nd_sb[:, b0 * 8:(b0 + nb) * 8],
                        nb * 128, nb * 128, xrowp)
                    if is_last:
                        x1g = fp.tile([128, Bh, cfg.xrowps[1]], bf16,
                                      tag="x1g")
                        nc.gpsimd.dma_gather(
                            x1g[:, :nb, :], xtab[1][:],
                            nd_sb[:, b0 * 8:(b0 + nb) * 8],
                            nb * 128, nb * 128, cfg.xrowps[1])

                    for b in range(b0, b0 + nb):
                        br = b - b0
                        hg = gp.tile([128, CH, rowp], bf16, tag="hg")
                        if li == 0 and b < 2:
                            nc.vector.memset(hg[:], 0.0)
                        # padding slots keep stale SBUF data; zero the alpha
                        # columns so exp() stays finite (S's zero columns
                        # cancel finite garbage exactly, but not Inf/NaN)
                        nc.vector.memset(hg[:, :, F:F + H], 0.0)
                        if "hg" in _abl:
                            nc.sync.dma_start(out=hg[:, 0, :],
                                              in_=hA_full[li][0:128, :])
                        else:
                            nc.gpsimd.reg_load(r_lo, gcnt_sb[0:1, 2*b:2*b+1])
                            nc.gpsimd.dma_gather(
                                hg[:, :LOCH, :], hA_full[li][0:cfg.losplit, :],
                                idx_sb[:, b * colw:
                                       b * colw + cfg.lo_cap // 16],
                                cfg.lo_cap, r_lo, rowp)
                            nc.gpsimd.reg_load(r_hi,
                                               gcnt_sb[0:1, 2*b+1:2*b+2])
                            nc.gpsimd.dma_gather(
                                hg[:, LOCH:, :], hA_full[li][cfg.losplit:N, :],
                                idx_sb[:, b * colw + cfg.lo_cap // 16:
                                       (b + 1) * colw],
                                cfg.hi_cap, r_hi, rowp)
                        S_sb = wp.tile([128, CH * 128], bf16, tag="S")
                        iota_b = bass.AP(iota_sb[:].tensor, iota_sb[:].offset,
                                         [iota_sb[:].ap[0], [0, CH], [1, 128]])
                        nc.vector.tensor_tensor(
                            out=S_sb[:].rearrange("p (c n) -> p c n", c=CH),
                            in0=dstloc_sb[:, b * CH:(b + 1) * CH]
                            .to_broadcast([128, CH, 128]),
                            in1=iota_b, op=mybir.AluOpType.is_equal)
                        # S^T precomputed at setup, one DMA per block
                        ST_sb = wp.tile([128, CH * 128], bf16, tag="ST")
                        nc.sync.dma_start(
                            out=ST_sb[:],
                            in_=ST_dram[:, b * CH * 128:(b + 1) * CH * 128])

                        # alpha_dst broadcast to edges: S^T_j @ ad_block
                        adps = pp.tile([128, CH * H], f32, tag="adps")
                        if "adps" in _abl:
                            nc.vector.memset(adps[:], 0.0)
                        else:
                            for j in range(CH):
                                nc.tensor.matmul(
                                    out=adps[:, j * H:(j + 1) * H],
                                    lhsT=ST_sb[:, j * 128:(j + 1) * 128],
                                    rhs=xad[:, br, adcol:adcol + H],
                                    start=True, stop=True)

                        # e = lrelu(alpha_s[src] + alpha_d[dst]); ex = exp(e)
                        et = sp.tile([128, CH * H], f32, tag="et")
                        nc.vector.tensor_tensor(
                            out=et[:].rearrange("p (c h) -> p c h", c=CH),
                            in0=hg[:, :, F:F + H],
                            in1=adps[:].rearrange("p (c h) -> p c h", c=CH),
                            op=mybir.AluOpType.add)
                        et2 = sp.tile([128, CH * H], f32, tag="et2")
                        nc.vector.tensor_scalar_mul(out=et2[:], in0=et[:],
                                                    scalar1=NEG_SLOPE)
                        nc.vector.tensor_tensor(out=et[:], in0=et[:],
                                                in1=et2[:],
                                                op=mybir.AluOpType.max)
                        ex = sp.tile([128, CH * H], bf16, tag="ex")
                        nc.scalar.activation(
                            out=ex[:], in_=et[:],
                            func=mybir.ActivationFunctionType.Exp)

                        # msg = [hg * ex | ex]
                        msg = gp.tile([128, CH, F + H], bf16, tag="msg")
                        ex3 = ex[:].rearrange("p (c h) -> p c h", c=CH)
                        if "msgmul" in _abl:
                            nc.vector.tensor_copy(out=msg[:, 0, :F],
                                                  in_=hg[:, 0, :F])
                        else:
                            nc.vector.tensor_tensor(
                            out=msg[:, :, :F].rearrange(
                                "p c (h k) -> p c h k", h=H),
                            in0=hg[:, :, :F].rearrange(
                                "p c (h k) -> p c h k", h=H),
                                in1=ex3.to_broadcast([128, CH, H, C]),
                                op=mybir.AluOpType.mult)
                        nc.vector.tensor_copy(out=msg[:, :, F:], in_=ex3)

                        # aggregate
                        pts = [pp.tile([128, w], f32, tag=f"agg{si}",
                                       name=f"agg{si}")
                               for si, (o, w) in enumerate(segs)]
                        _nmm = 1 if "mm" in _abl else CH
                        for j in range(_nmm):
                            lhsT = S_sb[:, j * 128:(j + 1) * 128]
                            for (o, w), pt in zip(segs, pts):
                                nc.tensor.matmul(out=pt[:], lhsT=lhsT,
                                                 rhs=msg[:, j, o:o + w],
                                                 start=(j == 0),
                                                 stop=(j == _nmm - 1))

                        # normalize + bias + relu
                        dseg = len(segs) - 1
                        dof = F - segs[dseg][0]
                        den = sp.tile([128, H], f32, tag="den")
                        nc.vector.tensor_scalar_add(
                            out=den[:], in0=pts[dseg][:, dof:dof + H],
                            scalar1=EPS)
                        rec = sp.tile([128, H], f32, tag="rec")
                        nc.vector.reciprocal(out=rec[:], in_=den[:])
                        xt = wp.tile([128, F], f32, tag="xt")
                        for si, (o, w) in enumerate(segs):
                            wF = min(w, F - o)
                            h0, nh = o // C, min(w, F - o) // C
                            nc.vector.tensor_tensor(
                                out=xt[:, o:o + wF].rearrange(
                                    "p (h k) -> p h k", h=nh),
                                in0=pts[si][:, :wF].rearrange(
                                    "p (h k) -> p h k", h=nh),
                                in1=rec[:, h0:h0 + nh].to_broadcast(
                                    [128, nh, C]),
                                op=mybir.AluOpType.mult)
                        nc.vector.tensor_tensor(out=xt[:], in0=xt[:],
                                                in1=brep_sb[li][:],
                                                op=mybir.AluOpType.add)
                        x_bf = wp.tile([128, F], bf16, tag="xbf")
                        nc.vector.tensor_scalar_max(out=x_bf[:], in0=xt[:],
                                                    scalar1=0.0)

                        nidx = nidx_sb[:, b:b + 1]
                        if not is_last:
                            nc.gpsimd.indirect_dma_start(
                                out=xtab[li + 1][:], in_=x_bf[:],
                                in_offset=None,
                                out_offset=bass.IndirectOffsetOnAxis(
                                    ap=nidx, axis=0))
                        elif os.environ.get("GAT_NO_FINAL"):
                            res = sp.tile([128, 1], f32, tag="res")
                            nc.vector.tensor_copy(out=res[:], in_=x_bf[:, 0:1])
                            nc.gpsimd.indirect_dma_start(
                                out=out_buf[:], in_=res[:], in_offset=None,
                                out_offset=bass.IndirectOffsetOnAxis(
                                    ap=nidx, axis=0))
                        else:
                            scratch = wp.tile([128, FTOT], bf16,
                                              tag="fscratch", bufs=1)
                            F01, F02 = cfg.Fs[0], cfg.Fs[0] + cfg.Fs[1]
                            nc.vector.tensor_tensor(
                                out=scratch[:, :F01],
                                in0=x1g[:, br, 0:cfg.Fs[0]],
                                in1=wf_sb[:, :F01], op=mybir.AluOpType.mult)
                            nc.vector.tensor_tensor(
                                out=scratch[:, F01:F02],
                                in0=xad[:, br, 0:cfg.Fs[1]],
                                in1=wf_sb[:, F01:F02], op=mybir.AluOpType.mult)
                            nc.vector.tensor_tensor(
                                out=scratch[:, F02:], in0=x_bf[:],
                                in1=wf_sb[:, F02:], op=mybir.AluOpType.mult)
                            acc = sp.tile([128, 1], f32, tag="acc")
                            nc.vector.reduce_sum(out=acc[:], in_=scratch[:],
                                                 axis=mybir.AxisListType.X)
                            res = sp.tile([128, 1], f32, tag="res")
                            nc.scalar.activation(
                                out=res[:], in_=acc[:],
                                func=mybir.ActivationFunctionType.Sigmoid,
                                bias=bf_sb[:, 0:1])
                            nc.gpsimd.indirect_dma_start(
                                out=out_buf[:], in_=res[:], in_offset=None,
                                out_offset=bass.IndirectOffsetOnAxis(
                                    ap=nidx, axis=0))

                # ------- phase B(l+1) + AllGather --------------------------
                if not is_last and li + 1 < _maxl:
                    lin = li + 1
                    Fn, Hn = cfg.Fs[lin], cfg.Hs[lin]
                    rowwn = cfg.rowws[lin]
                    nsegs = seg_split(Fn + 2 * Hn)
                    for t0 in range(0, shard, 128):
                        w = min(128, shard - t0)
                        phs = [pp.tile([128, w2], f32, tag=f"agg{si}",
                                       name=f"bh{si}")
                               for si, (o2, w2) in enumerate(nsegs)]
                        xin = iop.tile([128, F], bf16, tag="bx")
                        nc.sync.dma_start(out=xin[:w],
                                          in_=xtab[li + 1][t0:t0 + w, 0:F])
                        for fi in range(F // 128):
                            ptr = pp.tile([128, 128], bf16, tag="btr")
                            nc.tensor.transpose(
                                out=ptr[:],
                                in_=xin[:, fi * 128:(fi + 1) * 128],
                                identity=ident[:])
                            xTt = iop.tile([128, 128], bf16, tag="bxT")
                            nc.vector.tensor_copy(out=xTt[:], in_=ptr[:])
                            wa_t, ww = wa_sb[lin][fi]
                            for (o2, w2), ph2 in zip(nsegs, phs):
                                nc.tensor.matmul(
                                    out=ph2[:], lhsT=xTt[:],
                                    rhs=wa_t[:ww, o2:o2 + w2],
                                    start=(fi == 0),
                                    stop=(fi == F // 128 - 1))
                        hcp = iop.tile([128, rowwn], bf16, tag="bhcp")
                        for si, (o2, w2) in enumerate(nsegs):
                            wh = min(w2, rowwn - o2)
                            if wh > 0:
                                nc.vector.tensor_copy(
                                    out=hcp[:w, o2:o2 + wh],
                                    in_=phs[si][:w, :wh])
                        acp = sp.tile([128, Hn], bf16, tag="bacp")
                        dseg2 = len(nsegs) - 1
                        dof2 = rowwn - nsegs[dseg2][0]
                        nc.vector.tensor_copy(
                            out=acp[:w], in_=phs[dseg2][:w, dof2:dof2 + Hn])
                        nc.sync.dma_start(
                            out=hA_shard[lin][t0:t0 + w, :rowwn],
                            in_=hcp[:w])
                        nc.sync.dma_start(
                            out=xtab[lin][t0:t0 + w, F:F + Hn], in_=acp[:w])
                    if not os.environ.get("GAT_NO_AG"):
                        nc.gpsimd.collective_compute(
                            "AllGather", mybir.AluOpType.bypass,
                            replica_groups=[list(range(cfg.ncores))],
                            ins=[hA_shard[lin][:]], outs=[hA_full[lin][:]])

            nc.sync.dma_start(out=out[:, :], in_=out_buf[:shard, :])

    nc.compile()
    return nc


# ------------------------------------------------------------- host entry ----


def make_inputs(cfg, plan, x, W1, as1, ad1, b1, W2, as2, ad2, b2,
                W3, as3, ad3, b3, Wf, bf):
    x = np.asarray(x, np.float32)
    Wa = [fold_weights(W, a_s, a_d, H, C)
          for (W, a_s, a_d, (Fin, H, C)) in
          [(W1, as1, ad1, cfg.layers[0]), (W2, as2, ad2, cfg.layers[1]),
           (W3, as3, ad3, cfg.layers[2])]]
    shard = cfg.shard
    rows = np.concatenate(
        [np.asarray(Wf, np.float32).reshape(1, -1),
         np.asarray(b1, np.float32).reshape(1, -1),
         np.asarray(b2, np.float32).reshape(1, -1),
         np.asarray(b3, np.float32).reshape(1, -1),
         np.arange(128, dtype=np.float32).reshape(1, 128)],
        axis=1).astype(BF16)
    bf_words = np.asarray(bf, np.float32).reshape(1, 1).view(np.uint16) \
        .view(BF16)                                  # raw f32 bytes, not cast
    rows = np.concatenate([rows, bf_words.reshape(1, 2)], axis=1)
    common = {"rows": rows}
    wa0_flat = Wa[0].astype(BF16).reshape(1, -1)
    wr = [Wa[li].astype(BF16) for li in (1, 2)]
    nr = [w.shape[0] // cfg.ncores for w in wr]
    in_maps = []
    for c in range(cfg.ncores):
        pc = plan["cores"][c]
        m = dict(common)
        m.update(xTs=np.ascontiguousarray(
                     x[c * shard:(c + 1) * shard].T).astype(BF16),
                 wflat=np.concatenate(
                     [wa0_flat,
                      wr[0][c * nr[0]:(c + 1) * nr[0]].reshape(1, -1),
                      wr[1][c * nr[1]:(c + 1) * nr[1]].reshape(1, -1)],
                     axis=1),
                 idxall=np.concatenate([pc["idx16"], pc["nd16"]], axis=1),
                 nodeidx=pc["nodeidx"], gcnt=pc["gcnt"],
                 dstloc8=pc["dstloc8"])
        in_maps.append(m)
    return in_maps


_CACHE = {}


def _get_compiled(cfg, edge_index):
    key = hash(np.asarray(edge_index).tobytes())
    if key not in _CACHE:
        plan = make_plan(cfg, np.asarray(edge_index))
        nc = build_nc(cfg, plan["B"])
        # the program is immutable after compile(); cache its JSON so the
        # per-call jax lowering doesn't re-serialize 22k instructions
        js = nc.to_json_bytes()
        nc.to_json_bytes = lambda: js
        _CACHE.clear()
        _CACHE[key] = (plan, nc)
    return _CACHE[key]


def kernel(x, edge_index, JetRawPt, W1, as1, ad1, b1, W2, as2, ad2, b2,
           W3, as3, ad3, b3, Wf, bf):
    cfg = REAL_CFG
    plan, nc = _get_compiled(cfg, np.asarray(edge_index))
    in_maps = make_inputs(cfg, plan, x, W1, as1, ad1, b1, W2, as2, ad2, b2,
                          W3, as3, ad3, b3, Wf, bf)
    res = bass_utils.run_bass_kernel_spmd(nc, in_maps,
                                          core_ids=list(range(cfg.ncores)))
    return np.concatenate([res.results[c]["out"]
                           for c in range(cfg.ncores)], axis=0)



# revision 42
# speedup vs baseline: 1.0135x; 1.0135x over previous
"""GATNet (3-layer GAT + final linear) on 8 Trainium2 NeuronCores via Bass.

Graph/data-parallel layout (per sharding hint):
  - Nodes sharded by dst across 8 cores (6250/core).  Every core keeps a full
    replica of hA_l = [h_l | alpha_src_l] (bf16, rows padded to 256B stride)
    in DRAM; per-edge features are fetched with batched GPSIMD dma_gather
    (int16 indices, table split at row 32768 into lo/hi halves).
  - Per core, edges are grouped into B blocks (<=128 dst nodes each,
    <=LO_CAP lo-edges + <=HI_CAP hi-edges, fixed CH=LO+HI chunks of 128 edge
    slots).  The one-hot S [e x n] is built on device from per-slot dst ids
    (dstloc == iota); S^T for all blocks is precomputed once at setup (per
    2-block group: equality + 128x128 XBAR DMA transposes, streamed to
    DRAM) so the block loop needs a single DMA per block instead of a
    24-instruction transpose chain (the block loop is latency-bound at
    ~10us per dependent instruction, not bandwidth-bound).
    Aggregation = PE matmul psum[n,:] += S_j^T @ msg_j, where
    msg = [h[src]*exp(e) | exp(e)] so the same matmul yields the softmax
    denominator; normalization happens after aggregation (linearity).
  - alpha_dst broadcast to edges via S^T-matmul with the block's alpha_dst
    rows (fetched batched from the x-tables; the layer-0 table holds only
    the 8 alpha_dst_1 columns - raw x is never read through the gather).
  - Layer transition: x_l(shard) -> XBAR-transpose -> matmul with
    Wa = [W | W@a_src | W@a_dst]; AllGather(shard) -> full hA replica
    (Shared address space for the HBM-HBM collective fast path).
    Layer-1 hA is computed the same way from a per-core x^T shard; the
    alpha_dst_1 column of the x0 table is filled by the same loop.
  - Final linear+sigmoid fused into layer-3 epilogue; host concatenates the
    per-core [6250,1] output shards.

Host-side cost of the per-call path is minimized because the axon tunnel
moves ~75 MB/s and re-runs jax lowering per call:
  - payload is ~0.7 MB/core: gather indices ship as [16, .] int16 and are
    partition-replicated on device; W2/W3 ship row-sharded and are
    AllGathered; wf/bias/iota rows ship as [1, .] and are partition-doubled
    on device via SBUF->SBUF DMA; no dense one-hot matrices ship at all.
  - the persistent XLA compilation cache is enabled (else every call pays
    a ~1.2 s walrus re-verify) and the immutable post-compile BIR JSON is
    cached on the nc object (else every call re-serializes 20k
    instructions, ~0.15 s).
"""

import os
import numpy as np
import ml_dtypes

import jax

# Persistent XLA compilation cache: without it every run_bass_kernel_spmd
# call re-runs the walrus BIR verify/NEFF-packaging subprocess (~1.2 s).
for _k, _v in (("jax_compilation_cache_dir", "/tmp/jax_comp_cache"),
               ("jax_persistent_cache_min_compile_time_secs", 0),
               ("jax_persistent_cache_min_entry_size_bytes", -1)):
    try:
        jax.config.update(_k, _v)
    except Exception:
        pass

from concourse import bass, mybir, bacc
import concourse.tile as tile
from concourse import bass_utils
from concourse.masks import make_identity

BF16 = ml_dtypes.bfloat16
NEG_SLOPE = 0.2
EPS = 1e-16
I16_SPLIT = 32768


def rup(x, m):
    return (x + m - 1) // m * m


# ---------------------------------------------------------------- config ----


class Cfg:
    def __init__(self, N, ncores, layers, lo_chunks, hi_chunks):
        self.N = N
        self.ncores = ncores
        self.shard = N // ncores
        assert self.shard * ncores == N
        self.layers = layers                       # [(Fin, H, C)]
        self.loch, self.hich = lo_chunks, hi_chunks
        self.chunks = lo_chunks + hi_chunks
        self.lo_cap = lo_chunks * 128
        self.hi_cap = hi_chunks * 128
        self.losplit = I16_SPLIT if N > I16_SPLIT else N // 2
        self.Fs = [H * C for (_, H, C) in layers]
        self.Hs = [H for (_, H, C) in layers]
        self.rowws = [F + H for F, H in zip(self.Fs, self.Hs)]
        self.rowps = [rup(r, 128) for r in self.rowws]      # padded hA rows
        # x tables: [x_l | alpha_dst_{l+1}]; x0 table holds only alpha_dst_1
        # (the raw x columns are never read through the gather path)
        self.xrows = [self.Hs[0],
                      self.Fs[0] + self.Hs[1], self.Fs[1] + self.Hs[2]]
        self.xrowps = [rup(r, 128) for r in self.xrows]


REAL_CFG = Cfg(50000, 8, [(16, 8, 32), (256, 8, 32), (256, 12, 64)], 8, 4)


# ---------------------------------------------------------- host planning ----


def wrap16(vals, cap):
    """int16 idx stream -> wrapped [16, cap//16] layout (16 partitions).

    The gpsimd gather wants this replicated across all 8 16-partition
    groups; the replication happens on device (8 DMA loads) to keep the
    host->device payload small."""
    assert len(vals) == cap and cap % 16 == 0
    return np.asarray(vals, np.int16).reshape(cap // 16, 16).T  # [16, cap/16]


def make_plan(cfg, edge_index):
    N, shard, CH = cfg.N, cfg.shard, cfg.chunks
    src = np.concatenate([edge_index[0].astype(np.int64), np.arange(N)])
    dst = np.concatenate([edge_index[1].astype(np.int64), np.arange(N)])
    order = np.argsort(dst, kind="stable")
    src, dst = src[order].astype(np.int64), dst[order].astype(np.int64)

    bounds = np.searchsorted(dst, np.arange(0, N + 1, shard))
    is_lo = src < cfg.losplit
    deg_lo = np.bincount(dst[is_lo], minlength=N)
    deg_hi = np.bincount(dst[~is_lo], minlength=N)

    # greedy per-core blocks
    per_core_blocks = []
    for c in range(cfg.ncores):
        blocks, n = [], 0
        while n < shard:
            n_end, lo, hi = n, 0, 0
            while n_end < shard and n_end - n < 128:
                g = c * shard + n_end
                if lo + deg_lo[g] > cfg.lo_cap or hi + deg_hi[g] > cfg.hi_cap:
                    assert n_end > n, "single node exceeds caps"
                    break
                lo += deg_lo[g]
                hi += deg_hi[g]
                n_end += 1
            blocks.append((n, n_end))
            n = n_end
        per_core_blocks.append(blocks)
    B = max(len(b) for b in per_core_blocks)

    plan = {"B": B, "cores": []}
    for c in range(cfg.ncores):
        blocks = per_core_blocks[c] + \
            [(shard, shard)] * (B - len(per_core_blocks[c]))
        e0, e1 = bounds[c], bounds[c + 1]
        csrc = src[e0:e1]
        cdstl = dst[e0:e1] - c * shard
        node_starts = np.searchsorted(cdstl, np.arange(shard + 1))

        colw = CH * 128 // 16
        idx16 = np.zeros((16, B * colw), np.int16)
        nd16 = np.zeros((16, B * 8), np.int16)
        nodeidx = np.full((128, B), shard, np.int32)
        dl_int = np.full((128, B * CH), 128, np.int32)
        gcnt = np.full((1, 2 * B), 1, np.int32)
        for b, (n0, n1) in enumerate(blocks):
            nn = n1 - n0
            if nn > 0:
                nodeidx[:nn, b] = np.arange(n0, n1)
            ndstream = np.full(128, shard, np.int64)
            ndstream[:nn] = np.arange(n0, n1)
            nd16[:, b * 8:(b + 1) * 8] = wrap16(ndstream, 128)
            es, ee = node_starts[n0], node_starts[n1]
            bsrc, bdstl = csrc[es:ee], cdstl[es:ee]
            blo = bsrc < cfg.losplit
            lo_src, lo_dst = bsrc[blo], bdstl[blo]
            hi_src, hi_dst = bsrc[~blo] - cfg.losplit, bdstl[~blo]
            assert len(lo_src) <= cfg.lo_cap and len(hi_src) <= cfg.hi_cap
            lo_stream = np.full(cfg.lo_cap, -1, np.int64)
            lo_stream[:len(lo_src)] = lo_src
            hi_stream = np.full(cfg.hi_cap, -1, np.int64)
            hi_stream[:len(hi_src)] = hi_src
            if len(lo_src) == 0:
                lo_stream[0] = 0
            if len(hi_src) == 0:
                hi_stream[0] = 0
            gcnt[0, 2 * b] = max(len(lo_src), 1)
            gcnt[0, 2 * b + 1] = max(len(hi_src), 1)
            idx16[:, b * colw: b * colw + cfg.lo_cap // 16] = \
                wrap16(lo_stream, cfg.lo_cap)
            idx16[:, b * colw + cfg.lo_cap // 16:(b + 1) * colw] = \
                wrap16(hi_stream, cfg.hi_cap)
            # S: slot i -> (partition i%128, chunk i//128); built on device
            for sdst, base in [(lo_dst, 0), (hi_dst, cfg.lo_cap)]:
                ne = len(sdst)
                if ne == 0:
                    continue
                i = base + np.arange(ne)
                p, ch = i % 128, i // 128
                nl = (sdst - n0).astype(np.int64)
                dl_int[p, b * CH + ch] = nl
        plan["cores"].append(
            dict(idx16=idx16, nd16=nd16, nodeidx=nodeidx, gcnt=gcnt,
                 dstloc8=dl_int.astype(np.uint8)))
    return plan


def fold_weights(W, a_s, a_d, H, C):
    F = H * C
    Wr = np.asarray(W, np.float32).reshape(-1, H, C)
    ws = np.einsum("fhc,hc->fh", Wr, np.asarray(a_s, np.float32))
    wd = np.einsum("fhc,hc->fh", Wr, np.asarray(a_d, np.float32))
    return np.concatenate([Wr.reshape(Wr.shape[0], -1), ws, wd], axis=1)


def seg_split(total):
    segs, o = [], 0
    while o < total:
        w = min(512, total - o)
        segs.append((o, w))
        o += w
    return segs


# ------------------------------------------------------------ bass program ----


def build_nc(cfg, B):
    CH, N, shard = cfg.chunks, cfg.N, cfg.shard
    LOCH = cfg.loch
    dt = mybir.dt
    f32, bf16, i16, i32 = dt.float32, dt.bfloat16, dt.int16, dt.int32
    colw = CH * 128 // 16
    Bh = 8                     # blocks per fetch segment (<=1024 idxs/call)

    nc = bacc.Bacc("TRN2", target_bir_lowering=False, debug=False,
                   enable_asserts=False, num_devices=cfg.ncores)

    # ---- I/O ----
    Fin1 = cfg.layers[0][0]
    FTOT = sum(cfg.Fs)
    wrows = [cfg.layers[li][0] // cfg.ncores for li in range(3)]
    xTs = nc.dram_tensor("xTs", [Fin1, shard], bf16, kind="ExternalInput")
    Wa0_in = nc.dram_tensor("Wa0", [cfg.layers[0][0],
                                    cfg.Fs[0] + 2 * cfg.Hs[0]], bf16,
                            kind="ExternalInput")
    Waps = [None] + [nc.dram_tensor(f"Wa{li}p",
                                    [wrows[li], cfg.Fs[li] + 2 * cfg.Hs[li]],
                                    bf16, kind="ExternalInput")
                     for li in (1, 2)]
    # one packed row input: [wf | brep0 | brep1 | brep2 | iota]
    o_b0 = FTOT
    o_b1 = o_b0 + cfg.Fs[0]
    o_b2 = o_b1 + cfg.Fs[1]
    o_io = o_b2 + cfg.Fs[2]
    rows_in = nc.dram_tensor("rows", [1, o_io + 128], bf16,
                             kind="ExternalInput")
    bf_sc = nc.dram_tensor("bf_sc", [1, 1], f32, kind="ExternalInput")
    idxall_in = nc.dram_tensor("idxall", [16, B * colw + B * 8], i16,
                               kind="ExternalInput")
    nodeidx_in = nc.dram_tensor("nodeidx", [128, B], i32, kind="ExternalInput")
    gcnt_in = nc.dram_tensor("gcnt", [1, 2 * B], i32, kind="ExternalInput")
    dstloc8_in = nc.dram_tensor("dstloc8", [128, B * CH], dt.uint8,
                                kind="ExternalInput")
    out = nc.dram_tensor("out", [shard, 1], f32, kind="ExternalOutput")

    # ---- internal DRAM ----
    Wstg = [None] + [nc.dram_tensor(f"Wa{li}s",
                                    [wrows[li], cfg.Fs[li] + 2 * cfg.Hs[li]],
                                    bf16, kind="Internal") for li in (1, 2)]
    Was = [Wa0_in] + [nc.dram_tensor(f"Wa{li}f",
                                     [cfg.layers[li][0],
                                      cfg.Fs[li] + 2 * cfg.Hs[li]], bf16,
                                     kind="Internal", addr_space="Shared")
                      for li in (1, 2)]
    hA_full = [nc.dram_tensor(f"hAfull{li}", [N, cfg.rowps[li]], bf16,
                              kind="Internal", addr_space="Shared")
               for li in range(3)]
    hA_shard = [nc.dram_tensor(f"hAshard{li}", [shard, cfg.rowps[li]],
                               bf16, kind="Internal") for li in range(3)]
    xtab = [nc.dram_tensor(f"xtab{li}", [shard + 6, cfg.xrowps[li]],
                           bf16, kind="Internal") for li in range(3)]
    ST_dram = nc.dram_tensor("STd", [128, B * CH * 128], bf16,
                             kind="Internal")
    out_buf = nc.dram_tensor("out_buf", [shard + 1, 1], f32, kind="Internal")

    with tile.TileContext(nc) as tc:
        with tc.tile_pool(name="const", bufs=1) as cpool, \
             tc.tile_pool(name="io", bufs=3) as iop, \
             tc.tile_pool(name="gath", bufs=2) as gp, \
             tc.tile_pool(name="fetch", bufs=1) as fp, \
             tc.tile_pool(name="work", bufs=2) as wp, \
             tc.tile_pool(name="small", bufs=3) as sp, \
             tc.tile_pool(name="stsetup", bufs=2) as stp, \
             tc.tile_pool(name="psum", bufs=2, space="PSUM") as pp:


            ident = cpool.tile([128, 128], bf16)
            make_identity(nc, ident[:])

            # assemble full W2/W3 from the row shards (1/8 upload each);
            # collectives cannot read IO tensors, so stage via Internal DRAM
            for li in (1, 2):
                nc.sync.dma_start(out=Wstg[li][:, :], in_=Waps[li][:, :])
                nc.gpsimd.collective_compute(
                    "AllGather", mybir.AluOpType.bypass,
                    replica_groups=[list(range(cfg.ncores))],
                    ins=[Wstg[li][:]], outs=[Was[li][:]])

            def bcast_rows(t, row_in, width):
                """fill t[0:128, :width] with row_in via log2 partition
                doubling (SBUF->SBUF DMA)."""
                nc.sync.dma_start(out=t[0:1, :width], in_=row_in)
                p = 1
                while p < 128:
                    nc.sync.dma_start(out=t[p:2 * p, :width],
                                      in_=t[0:p, :width])
                    p *= 2

            wa_sb = []
            for li in range(3):
                Fin, H = cfg.layers[li][0], cfg.Hs[li]
                tiles = []
                for f0 in range(0, Fin, 128):
                    w = min(128, Fin - f0)
                    t = cpool.tile([128, cfg.Fs[li] + 2 * H], bf16,
                                   tag=f"wa{li}_{f0}", name=f"wa{li}_{f0}")
                    nc.sync.dma_start(out=t[:w], in_=Was[li][f0:f0 + w, :])
                    tiles.append((t, w))
                wa_sb.append(tiles)
            brep_sb = []
            boffs = [o_b0, o_b1, o_b2]
            for li in range(3):
                t = cpool.tile([128, cfg.Fs[li]], bf16, tag=f"brep{li}",
                               name=f"brepsb{li}")
                bcast_rows(t, rows_in[0:1, boffs[li]:boffs[li] + cfg.Fs[li]],
                           cfg.Fs[li])
                brep_sb.append(t)
            wf_sb = cpool.tile([128, FTOT], bf16)
            bcast_rows(wf_sb, rows_in[0:1, 0:FTOT], FTOT)
            bf_sb = cpool.tile([128, 1], f32)
            bcast_rows(bf_sb, bf_sc[0:1, :], 1)
            iota_sb = cpool.tile([128, 128], bf16)
            bcast_rows(iota_sb, rows_in[0:1, o_io:o_io + 128], 128)
            idx_sb = cpool.tile([128, B * colw], i16)
            nd_sb = cpool.tile([128, B * 8], i16)
            for g in range(8):
                nc.sync.dma_start(out=idx_sb[16 * g:16 * (g + 1), :],
                                  in_=idxall_in[:, :B * colw])
                nc.sync.dma_start(out=nd_sb[16 * g:16 * (g + 1), :],
                                  in_=idxall_in[:, B * colw:])
            nidx_sb = cpool.tile([128, B], i32)
            nc.sync.dma_start(out=nidx_sb[:], in_=nodeidx_in[:, :])
            dl8_sb = cpool.tile([128, B * CH], dt.uint8)
            nc.sync.dma_start(out=dl8_sb[:], in_=dstloc8_in[:, :])
            dstloc_sb = cpool.tile([128, B * CH], bf16)
            nc.vector.tensor_copy(out=dstloc_sb[:], in_=dl8_sb[:])
            gcnt_sb = cpool.tile([1, 2 * B], i32)
            nc.sync.dma_start(out=gcnt_sb[:], in_=gcnt_in[:, :])
            r_lo = nc.gpsimd.alloc_register("r_lo")
            r_hi = nc.gpsimd.alloc_register("r_hi")

            # build S^T for every block once: S = (dstloc == iota) per
            # block, 128x128 XBAR DMA transposes, stream to DRAM; double-
            # buffered so group g+1's equality overlaps g's transposes
            for q0 in range(B):
                Sq = stp.tile([128, CH * 128], bf16, tag="Sq")
                iota_q = bass.AP(iota_sb[:].tensor, iota_sb[:].offset,
                                 [iota_sb[:].ap[0], [0, CH], [1, 128]])
                nc.vector.tensor_tensor(
                    out=Sq[:].rearrange("p (c n) -> p c n", c=CH),
                    in0=dstloc_sb[:, q0 * CH:(q0 + 1) * CH]
                    .to_broadcast([128, CH, 128]),
                    in1=iota_q, op=mybir.AluOpType.is_equal)
                STq = stp.tile([128, CH * 128], bf16, tag="STq")
                for j in range(CH):
                    nc.sync.dma_start(out=STq[:, j * 128:(j + 1) * 128],
                                      in_=Sq[:, j * 128:(j + 1) * 128],
                                      transpose=True)
                nc.sync.dma_start(
                    out=ST_dram[:, q0 * CH * 128:(q0 + 1) * CH * 128],
                    in_=STq[:])

            # zero dummy rows of internal x tables
            zrow = cpool.tile([1, 512], bf16)
            nc.vector.memset(zrow[:], 0.0)
            for li in (0, 1, 2):
                nc.sync.dma_start(out=xtab[li][shard:shard + 1, :],
                                  in_=zrow[:1, :cfg.xrowps[li]])

            # ------- phase B1: hA1 = x @ Wa1 for own shard + AllGather -------
            roww0 = cfg.rowws[0]
            H1 = cfg.Hs[0]
            wa1_t = wa_sb[0][0][0]
            for t0 in range(0, shard, 128):
                w = min(128, shard - t0)
                lhs = iop.tile([Fin1, 128], bf16, tag="b1lhs")
                if w < 128:
                    nc.vector.memset(lhs[:], 0.0)
                nc.sync.dma_start(out=lhs[:, :w], in_=xTs[:, t0:t0 + w])
                ph = pp.tile([128, cfg.Fs[0] + 2 * cfg.Hs[0]], f32, tag="agg0")
                nc.tensor.matmul(out=ph[:], lhsT=lhs[:], rhs=wa1_t[:Fin1],
                                 start=True, stop=True)
                hcp = iop.tile([128, roww0], bf16, tag="b1h")
                nc.vector.tensor_copy(out=hcp[:w], in_=ph[:w, :roww0])
                nc.sync.dma_start(out=hA_shard[0][t0:t0 + w, :roww0],
                                  in_=hcp[:w])
                acp = sp.tile([128, H1], bf16, tag="bacp")
                nc.vector.tensor_copy(out=acp[:w],
                                      in_=ph[:w, roww0:roww0 + H1])
                nc.sync.dma_start(out=xtab[0][t0:t0 + w, 0:H1], in_=acp[:w])
            if not os.environ.get("GAT_NO_AG"):
                nc.gpsimd.collective_compute(
                    "AllGather", mybir.AluOpType.bypass,
                    replica_groups=[list(range(cfg.ncores))],
                    ins=[hA_shard[0][:]], outs=[hA_full[0][:]])

            # ---------------- layers ----------------------------------------
            _abl = set(os.environ.get("GAT_ABL", "").split(","))
            _maxl = int(os.environ.get("GAT_LAYERS", "3"))
            for li in range(_maxl):
                Fin, H, C = cfg.layers[li]
                F, rowp = cfg.Fs[li], cfg.rowps[li]
                segs = seg_split(F + H)
                is_last = li == 2
                adcol = 0 if li == 0 else cfg.Fs[li - 1]     # alpha_dst col
                xrowp = cfg.xrowps[li]

                for half in range((B + Bh - 1) // Bh):
                    b0 = half * Bh
                    nb = min(Bh, B - b0)
                    if nb <= 0:
                        continue
                    xad = fp.tile([128, Bh, xrowp], bf16, tag="xad")
                    nc.gpsimd.dma_gather(
                        xad[:, :nb, :], xtab[li][:],
                        nd_sb[:, b0 * 8:(b0 + nb) * 8],
                        nb * 128, nb * 128, xrowp)
                    if is_last:
                        x1g = fp.tile([128, Bh, cfg.xrowps[1]], bf16,
                                      tag="x1g")
                        nc.gpsimd.dma_gather(
                            x1g[:, :nb, :], xtab[1][:],
                            nd_sb[:, b0 * 8:(b0 + nb) * 8],
                            nb * 128, nb * 128, cfg.xrowps[1])

                    for b in range(b0, b0 + nb):
                        br = b - b0
                        hg = gp.tile([128, CH, rowp], bf16, tag="hg")
                        if li == 0 and b < 2:
                            nc.vector.memset(hg[:], 0.0)
                        # padding slots keep stale SBUF data; zero the alpha
                        # columns so exp() stays finite (S's zero columns
                        # cancel finite garbage exactly, but not Inf/NaN)
                        nc.vector.memset(hg[:, :, F:F + H], 0.0)
                        if "hg" in _abl:
                            nc.sync.dma_start(out=hg[:, 0, :],
                                              in_=hA_full[li][0:128, :])
                        else:
                            nc.gpsimd.reg_load(r_lo, gcnt_sb[0:1, 2*b:2*b+1])
                            nc.gpsimd.dma_gather(
                                hg[:, :LOCH, :], hA_full[li][0:cfg.losplit, :],
                                idx_sb[:, b * colw:
                                       b * colw + cfg.lo_cap // 16],
                                cfg.lo_cap, r_lo, rowp)
                            nc.gpsimd.reg_load(r_hi,
                                               gcnt_sb[0:1, 2*b+1:2*b+2])
                            nc.gpsimd.dma_gather(
                                hg[:, LOCH:, :], hA_full[li][cfg.losplit:N, :],
                                idx_sb[:, b * colw + cfg.lo_cap // 16:
                                       (b + 1) * colw],
                                cfg.hi_cap, r_hi, rowp)
                        S_sb = wp.tile([128, CH * 128], bf16, tag="S")
                        iota_b = bass.AP(iota_sb[:].tensor, iota_sb[:].offset,
                                         [iota_sb[:].ap[0], [0, CH], [1, 128]])
                        nc.vector.tensor_tensor(
                            out=S_sb[:].rearrange("p (c n) -> p c n", c=CH),
                            in0=dstloc_sb[:, b * CH:(b + 1) * CH]
                            .to_broadcast([128, CH, 128]),
                            in1=iota_b, op=mybir.AluOpType.is_equal)
                        # S^T precomputed at setup, one DMA per block
                        ST_sb = wp.tile([128, CH * 128], bf16, tag="ST")
                        nc.sync.dma_start(
                            out=ST_sb[:],
                            in_=ST_dram[:, b * CH * 128:(b + 1) * CH * 128])

                        # alpha_dst broadcast to edges: S^T_j @ ad_block
                        adps = pp.tile([128, CH * H], f32, tag="adps")
                        if "adps" in _abl:
                            nc.vector.memset(adps[:], 0.0)
                        else:
                            for j in range(CH):
                                nc.tensor.matmul(
                                    out=adps[:, j * H:(j + 1) * H],
                                    lhsT=ST_sb[:, j * 128:(j + 1) * 128],
                                    rhs=xad[:, br, adcol:adcol + H],
                                    start=True, stop=True)

                        # e = lrelu(alpha_s[src] + alpha_d[dst]); ex = exp(e)
                        et = sp.tile([128, CH * H], f32, tag="et")
                        nc.vector.tensor_tensor(
                            out=et[:].rearrange("p (c h) -> p c h", c=CH),
                            in0=hg[:, :, F:F + H],
                            in1=adps[:].rearrange("p (c h) -> p c h", c=CH),
                            op=mybir.AluOpType.add)
                        et2 = sp.tile([128, CH * H], f32, tag="et2")
                        nc.vector.tensor_scalar_mul(out=et2[:], in0=et[:],
                                                    scalar1=NEG_SLOPE)
                        nc.vector.tensor_tensor(out=et[:], in0=et[:],
                                                in1=et2[:],
                                                op=mybir.AluOpType.max)
                        ex = sp.tile([128, CH * H], bf16, tag="ex")
                        nc.scalar.activation(
                            out=ex[:], in_=et[:],
                            func=mybir.ActivationFunctionType.Exp)

                        # msg = [hg * ex | ex]
                        msg = gp.tile([128, CH, F + H], bf16, tag="msg")
                        ex3 = ex[:].rearrange("p (c h) -> p c h", c=CH)
                        if "msgmul" in _abl:
                            nc.vector.tensor_copy(out=msg[:, 0, :F],
                                                  in_=hg[:, 0, :F])
                        else:
                            nc.vector.tensor_tensor(
                            out=msg[:, :, :F].rearrange(
                                "p c (h k) -> p c h k", h=H),
                            in0=hg[:, :, :F].rearrange(
                                "p c (h k) -> p c h k", h=H),
                                in1=ex3.to_broadcast([128, CH, H, C]),
                                op=mybir.AluOpType.mult)
                        nc.vector.tensor_copy(out=msg[:, :, F:], in_=ex3)

                        # aggregate
                        pts = [pp.tile([128, w], f32, tag=f"agg{si}",
                                       name=f"agg{si}")
                               for si, (o, w) in enumerate(segs)]
                        _nmm = 1 if "mm" in _abl else CH
                        for j in range(_nmm):
                            lhsT = S_sb[:, j * 128:(j + 1) * 128]
                            for (o, w), pt in zip(segs, pts):
                                nc.tensor.matmul(out=pt[:], lhsT=lhsT,
                                                 rhs=msg[:, j, o:o + w],
                                                 start=(j == 0),
                                                 stop=(j == _nmm - 1))

                        # normalize + bias + relu
                        dseg = len(segs) - 1
                        dof = F - segs[dseg][0]
                        den = sp.tile([128, H], f32, tag="den")
                        nc.vector.tensor_scalar_add(
                            out=den[:], in0=pts[dseg][:, dof:dof + H],
                            scalar1=EPS)
                        rec = sp.tile([128, H], f32, tag="rec")
                        nc.vector.reciprocal(out=rec[:], in_=den[:])
                        xt = wp.tile([128, F], f32, tag="xt")
                        for si, (o, w) in enumerate(segs):
                            wF = min(w, F - o)
                            h0, nh = o // C, min(w, F - o) // C
                            nc.vector.tensor_tensor(
                                out=xt[:, o:o + wF].rearrange(
                                    "p (h k) -> p h k", h=nh),
                                in0=pts[si][:, :wF].rearrange(
                                    "p (h k) -> p h k", h=nh),
                                in1=rec[:, h0:h0 + nh].to_broadcast(
                                    [128, nh, C]),
                                op=mybir.AluOpType.mult)
                        nc.vector.tensor_tensor(out=xt[:], in0=xt[:],
                                                in1=brep_sb[li][:],
                                                op=mybir.AluOpType.add)
                        x_bf = wp.tile([128, F], bf16, tag="xbf")
                        nc.vector.tensor_scalar_max(out=x_bf[:], in0=xt[:],
                                                    scalar1=0.0)

                        nidx = nidx_sb[:, b:b + 1]
                        if not is_last:
                            nc.gpsimd.indirect_dma_start(
                                out=xtab[li + 1][:], in_=x_bf[:],
                                in_offset=None,
                                out_offset=bass.IndirectOffsetOnAxis(
                                    ap=nidx, axis=0))
                        elif os.environ.get("GAT_NO_FINAL"):
                            res = sp.tile([128, 1], f32, tag="res")
                            nc.vector.tensor_copy(out=res[:], in_=x_bf[:, 0:1])
                            nc.gpsimd.indirect_dma_start(
                                out=out_buf[:], in_=res[:], in_offset=None,
                                out_offset=bass.IndirectOffsetOnAxis(
                                    ap=nidx, axis=0))
                        else:
                            scratch = wp.tile([128, FTOT], bf16,
                                              tag="fscratch", bufs=1)
                            F01, F02 = cfg.Fs[0], cfg.Fs[0] + cfg.Fs[1]
                            nc.vector.tensor_tensor(
                                out=scratch[:, :F01],
                                in0=x1g[:, br, 0:cfg.Fs[0]],
                                in1=wf_sb[:, :F01], op=mybir.AluOpType.mult)
                            nc.vector.tensor_tensor(
                                out=scratch[:, F01:F02],
                                in0=xad[:, br, 0:cfg.Fs[1]],
                                in1=wf_sb[:, F01:F02], op=mybir.AluOpType.mult)
                            nc.vector.tensor_tensor(
                                out=scratch[:, F02:], in0=x_bf[:],
                                in1=wf_sb[:, F02:], op=mybir.AluOpType.mult)
                            acc = sp.tile([128, 1], f32, tag="acc")
                            nc.vector.reduce_sum(out=acc[:], in_=scratch[:],
                                                 axis=mybir.AxisListType.X)
                            res = sp.tile([128, 1], f32, tag="res")
                            nc.scalar.activation(
                                out=res[:], in_=acc[:],
                                func=mybir.ActivationFunctionType.Sigmoid,
                                bias=bf_sb[:, 0:1])
                            nc.gpsimd.indirect_dma_start(
                                out=out_buf[:], in_=res[:], in_offset=None,
                                out_offset=bass.IndirectOffsetOnAxis(
                                    ap=nidx, axis=0))

                # ------- phase B(l+1) + AllGather --------------------------
                if not is_last and li + 1 < _maxl:
                    lin = li + 1
                    Fn, Hn = cfg.Fs[lin], cfg.Hs[lin]
                    rowwn = cfg.rowws[lin]
                    nsegs = seg_split(Fn + 2 * Hn)
                    for t0 in range(0, shard, 128):
                        w = min(128, shard - t0)
                        phs = [pp.tile([128, w2], f32, tag=f"agg{si}",
                                       name=f"bh{si}")
                               for si, (o2, w2) in enumerate(nsegs)]
                        xin = iop.tile([128, F], bf16, tag="bx")
                        nc.sync.dma_start(out=xin[:w],
                                          in_=xtab[li + 1][t0:t0 + w, 0:F])
                        for fi in range(F // 128):
                            ptr = pp.tile([128, 128], bf16, tag="btr")
                            nc.tensor.transpose(
                                out=ptr[:],
                                in_=xin[:, fi * 128:(fi + 1) * 128],
                                identity=ident[:])
                            xTt = iop.tile([128, 128], bf16, tag="bxT")
                            nc.vector.tensor_copy(out=xTt[:], in_=ptr[:])
                            wa_t, ww = wa_sb[lin][fi]
                            for (o2, w2), ph2 in zip(nsegs, phs):
                                nc.tensor.matmul(
                                    out=ph2[:], lhsT=xTt[:],
                                    rhs=wa_t[:ww, o2:o2 + w2],
                                    start=(fi == 0),
                                    stop=(fi == F // 128 - 1))
                        hcp = iop.tile([128, rowwn], bf16, tag="bhcp")
                        for si, (o2, w2) in enumerate(nsegs):
                            wh = min(w2, rowwn - o2)
                            if wh > 0:
                                nc.vector.tensor_copy(
                                    out=hcp[:w, o2:o2 + wh],
                                    in_=phs[si][:w, :wh])
                        acp = sp.tile([128, Hn], bf16, tag="bacp")
                        dseg2 = len(nsegs) - 1
                        dof2 = rowwn - nsegs[dseg2][0]
                        nc.vector.tensor_copy(
                            out=acp[:w], in_=phs[dseg2][:w, dof2:dof2 + Hn])
                        nc.sync.dma_start(
                            out=hA_shard[lin][t0:t0 + w, :rowwn],
                            in_=hcp[:w])
                        nc.sync.dma_start(
                            out=xtab[lin][t0:t0 + w, F:F + Hn], in_=acp[:w])
                    if not os.environ.get("GAT_NO_AG"):
                        nc.gpsimd.collective_compute(
                            "AllGather", mybir.AluOpType.bypass,
                            replica_groups=[list(range(cfg.ncores))],
                            ins=[hA_shard[lin][:]], outs=[hA_full[lin][:]])

            nc.sync.dma_start(out=out[:, :], in_=out_buf[:shard, :])

    nc.compile()
    return nc


# ------------------------------------------------------------- host entry ----


def make_inputs(cfg, plan, x, W1, as1, ad1, b1, W2, as2, ad2, b2,
                W3, as3, ad3, b3, Wf, bf):
    x = np.asarray(x, np.float32)
    Wa = [fold_weights(W, a_s, a_d, H, C)
          for (W, a_s, a_d, (Fin, H, C)) in
          [(W1, as1, ad1, cfg.layers[0]), (W2, as2, ad2, cfg.layers[1]),
           (W3, as3, ad3, cfg.layers[2])]]
    shard = cfg.shard
    rows = np.concatenate(
        [np.asarray(Wf, np.float32).reshape(1, -1),
         np.asarray(b1, np.float32).reshape(1, -1),
         np.asarray(b2, np.float32).reshape(1, -1),
         np.asarray(b3, np.float32).reshape(1, -1),
         np.arange(128, dtype=np.float32).reshape(1, 128)],
        axis=1).astype(BF16)
    common = {"Wa0": Wa[0].astype(BF16), "rows": rows,
              "bf_sc": np.asarray(bf, np.float32).reshape(1, 1)}
    wr = [Wa[li].astype(BF16) for li in (1, 2)]
    nr = [w.shape[0] // cfg.ncores for w in wr]
    in_maps = []
    for c in range(cfg.ncores):
        pc = plan["cores"][c]
        m = dict(common)
        m.update(xTs=np.ascontiguousarray(
                     x[c * shard:(c + 1) * shard].T).astype(BF16),
                 Wa1p=wr[0][c * nr[0]:(c + 1) * nr[0]].copy(),
                 Wa2p=wr[1][c * nr[1]:(c + 1) * nr[1]].copy(),
                 idxall=np.concatenate([pc["idx16"], pc["nd16"]], axis=1),
                 nodeidx=pc["nodeidx"], gcnt=pc["gcnt"],
                 dstloc8=pc["dstloc8"])
        in_maps.append(m)
    return in_maps


_CACHE = {}


def _get_compiled(cfg, edge_index):
    key = hash(np.asarray(edge_index).tobytes())
    if key not in _CACHE:
        plan = make_plan(cfg, np.asarray(edge_index))
        nc = build_nc(cfg, plan["B"])
        # the program is immutable after compile(); cache its JSON so the
        # per-call jax lowering doesn't re-serialize 22k instructions
        js = nc.to_json_bytes()
        nc.to_json_bytes = lambda: js
        _CACHE.clear()
        _CACHE[key] = (plan, nc)
    return _CACHE[key]


def kernel(x, edge_index, JetRawPt, W1, as1, ad1, b1, W2, as2, ad2, b2,
           W3, as3, ad3, b3, Wf, bf):
    cfg = REAL_CFG
    plan, nc = _get_compiled(cfg, np.asarray(edge_index))
    in_maps = make_inputs(cfg, plan, x, W1, as1, ad1, b1, W2, as2, ad2, b2,
                          W3, as3, ad3, b3, Wf, bf)
    res = bass_utils.run_bass_kernel_spmd(nc, in_maps,
                                          core_ids=list(range(cfg.ncores)))
    return np.concatenate([res.results[c]["out"]
                           for c in range(cfg.ncores)], axis=0)

